# revision 23
# baseline (speedup 1.0000x reference)
import sys

sys.path.insert(0, "/opt/trn_rl_repo")

import numpy as np
import ml_dtypes
from concourse import bass, bacc, tile, bass_utils
from concourse.bass import mybir
from concourse.dve_spec import Spec, Src0, Src1, C0, lower as dve_lower, maxx
import concourse.dve_ops as dve_ops
from concourse.dve_uop import DveOpSpec

# Problem: queries (8, 2048, 512) f32, items (4096, 512) f32 -> (8, 2048) f32
#   score = q @ items.T ; j = argmax_m score[t, m] (softmax+top2 reduces to this)
#   out[t] = -score[t, j] / (||q_t|| * ||items_j||)
# Sharding: batch row b -> core b. Per core: T=2048 tokens, M=4096 items, C=512.
#
# Device pipeline per 128-token tile:
#   - matmuls accumulate scores into two 4-bank PSUM half-tiles
#     (fp8 e4m3 DoubleRow at 2 rows/cycle, or fp16 at 1 row/cycle).
#   - One custom DVE op per half reads PSUM directly:
#       out   = ((s + C0) - C0) + njit[m]      # quantize s to 2^-4 grid
#       accum = max(out)                        # packed running max
#     C0=3*2^18 pins the fp32 exponent so the +/- C0 round-trip snaps s onto a
#     uniform 2^-4 grid (Sterbenz-exact subtract); njit[m] = code[m]*2^-16
#     (code <= 2046 = quantized inverse item norm) rides in the freed low
#     mantissa bits; the packed sum is exactly representable in fp32.
#   - Tail decodes winner score and norm code from the single accumulated
#     value per token: out = -s_q * (r_lo + c1*code) / ||q||.
# No PSUM->SBUF copy pass exists; ScalarE is idle. Score-grid ties resolve
# deterministically to the largest norm code.

NCORES = 8
T = 2048
C = 512
M = 4096
NT = T // 128   # 16 token tiles
NB = M // 512   # 8 psum banks of 512 items
SPLIT = 1       # 0: fp16 matmul, 1: fp8 e4m3 DoubleRow matmul
KCH = 4         # contraction chunks of 128

F32 = mybir.dt.float32
F16 = mybir.dt.float16
F8 = mybir.dt.float8e4
OP = mybir.AluOpType
DR = mybir.MatmulPerfMode.DoubleRow

EPSC = 2.0 ** -16
NCODE = 2046
C0V = float(3 * 2 ** 18)

NP_F8 = ml_dtypes.float8_e4m3
IN_DT = F8 if SPLIT == 1 else F16
NP_IN = NP_F8 if SPLIT == 1 else np.float16


def _register_pack_max2():
    # accum_out = max over free dim of (grid-quantized in0 + in1).
    # Registered as a custom-DVE micro-op program carried in the NEFF DVE
    # table (the stock TENSOR_TENSOR_REDUCE opcode is fatal on this runtime).
    name = "PACK_MAX2_ANT"
    for existing in dve_ops.OPS:
        if existing.name == name:
            return existing

    def _ref(in0, in1, c0, c1, c2):
        q = (in0.astype(np.float32) + np.float32(c0)).astype(np.float32)
        q = (q - np.float32(c0)).astype(np.float32)
        o = q + in1.astype(np.float32)
        return o, np.maximum.reduce(o, axis=-1)

    spec = Spec(body=((Src0 + C0) - C0) + Src1, accum=maxx, reference=_ref)
    op = dve_ops.DveOp(name, spec, subdim=False, uops_sha={})
    dve_ops.OPS.append(op)
    dve_ops._SUB_OPCODE_FOR_NAME[name] = (
        dve_ops._CUSTOM_DVE_ROW_BASE + len(dve_ops.OPS) - 1
    )
    dve_ops.CUSTOM_DVE_SPECS[name] = spec
    for ver in ("v3", "v4"):
        uops = dve_lower(spec, ver=ver)
        op.uops_sha[ver] = DveOpSpec(
            name=name, opcode=dve_ops.get_dve_sub_opcode(name),
            uops=uops, rd1_en=True,
        ).sha(ver)
    return op


PACK_MAX2 = _register_pack_max2()


def _build(r_lo, c1):
    nc = bacc.Bacc()
    # tile-major q / half-major items: every DMA below moves contiguous
    # 512B-16KB runs per partition (few descriptors, low latency).
    q_d = nc.dram_tensor("q8", [128, NT, KCH, 128], IN_DT, kind="ExternalInput")
    i_d = nc.dram_tensor("i8", [128, 2, KCH, 2048], IN_DT, kind="ExternalInput")
    nj_d = nc.dram_tensor("nj", [128, M], F16, kind="ExternalInput")
    rq_d = nc.dram_tensor("rq", [128, NT], F32, kind="ExternalInput")
    out_d = nc.dram_tensor("out", [128, NT], F32, kind="ExternalOutput")

    with tile.TileContext(nc) as tc:
        with tc.tile_pool(name="big", bufs=1) as big, \
             tc.tile_pool(name="small", bufs=1) as small:

            it = big.tile([128, 2, KCH, 2048], IN_DT, name="it")
            qt = big.tile([128, NT, KCH, 128], IN_DT, name="qt")
            nj = big.tile([128, M], F16, name="nj")
            rq = small.tile([128, NT], F32, name="rq")
            dummy = small.tile([128, 1], F32, name="dummy")
            MA = small.tile([128, NT], F32, name="MA")
            MB = small.tile([128, NT], F32, name="MB")

            # DMA order: tile-0 q, items half 0 split by chunk pair
            # (kp0 matmuls start after the first 512KB), nj lo, items
            # half 1, nj hi, remaining q tiles
            nc.sync.dma_start(out=qt[:, 0], in_=q_d[:, 0])
            nc.sync.dma_start(out=it[:, 0, 0:2], in_=i_d[:, 0, 0:2])
            nc.sync.dma_start(out=it[:, 0, 2:4], in_=i_d[:, 0, 2:4])
            nc.sync.dma_start(out=nj[:, 0:2048], in_=nj_d[:, 0:2048])
            nc.sync.dma_start(out=it[:, 1], in_=i_d[:, 1])
            nc.sync.dma_start(out=nj[:, 2048:4096], in_=nj_d[:, 2048:4096])
            for i in range(1, NT):
                nc.sync.dma_start(out=qt[:, i], in_=q_d[:, i])
            nc.sync.dma_start(out=rq, in_=rq_d[:, :])

            with tc.tile_pool(name="bps", bufs=1, space="PSUM") as bps:
                # PE warmup: ~3.5us of throwaway matmuls issued while input
                # DMAs land, so the HAM clock-gate reaches 8/8 (2.4 GHz)
                # before the first real matmul. start=True on real tile-0
                # matmuls discards the garbage.
                wr = small.tile([128, 2, 512], IN_DT, name="wr")
                nc.gpsimd.memset(wr, 0)
                wl = wr[:, :, 0:128]
                warm = [
                    bps.tile([128, 2048], F32, tag=f"ps{h}", name="ps")
                    for h in range(2)
                ]
                for w in range(8):
                    nc.tensor.matmul(
                        warm[w // 4][:, bass.ts(w % 4, 512)],
                        wl if SPLIT == 1 else wl[:, 0, :],
                        wr[:, :, :] if SPLIT == 1 else wr[:, 0, :],
                        start=True, stop=True,
                        perf_mode=DR if SPLIT == 1 else None,
                    )
                for i in range(NT):
                    halves = [
                        bps.tile([128, 2048], F32, tag=f"ps{h}", name="ps")
                        for h in range(2)
                    ]
                    # half-A completes before half-B starts so pack-A can
                    # begin as early as possible
                    if SPLIT == 1:
                        for h in range(2):
                            for kp in range(KCH // 2):
                                for b in range(4):
                                    nc.tensor.matmul(
                                        halves[h][:, bass.ts(b, 512)],
                                        qt[:, i, 2 * kp:2 * kp + 2, :],
                                        it[:, h, 2 * kp:2 * kp + 2,
                                           bass.ts(b, 512)],
                                        start=(kp == 0),
                                        stop=(kp == KCH // 2 - 1),
                                        perf_mode=DR,
                                    )
                    else:
                        for h in range(2):
                            for k in range(KCH):
                                for b in range(4):
                                    nc.tensor.matmul(
                                        halves[h][:, bass.ts(b, 512)],
                                        qt[:, i, k, :],
                                        it[:, h, k, bass.ts(b, 512)],
                                        start=(k == 0), stop=(k == KCH - 1),
                                    )
                    nc.vector._custom_dve(
                        PACK_MAX2, out=dummy.broadcast_to([128, 2048]),
                        in0=halves[0], in1=nj[:, 0:2048], s0=C0V,
                        accum_out=MA[:, i:i + 1],
                    )
                    nc.vector._custom_dve(
                        PACK_MAX2, out=dummy.broadcast_to([128, 2048]),
                        in0=halves[1], in1=nj[:, 2048:4096], s0=C0V,
                        accum_out=MB[:, i:i + 1],
                    )

            # Decode: Mv = max(MA, MB); s_q = (Mv+C0)-C0; code = (Mv-s_q)*2^16
            # out = -s_q * (r_lo + c1*code) / ||q||
            Mv = small.tile([128, NT], F32, name="Mv")
            sq = small.tile([128, NT], F32, name="sq")
            dlt = small.tile([128, NT], F32, name="dlt")
            rhat = small.tile([128, NT], F32, name="rhat")
            t1 = small.tile([128, NT], F32, name="t1")
            outv = small.tile([128, NT], F32, name="outv")
            nc.vector.tensor_tensor(out=Mv, in0=MA, in1=MB, op=OP.max)
            nc.vector.tensor_scalar(
                out=sq, in0=Mv, scalar1=C0V, scalar2=C0V,
                op0=OP.add, op1=OP.subtract,
            )
            nc.vector.tensor_sub(dlt, Mv, sq)
            nc.vector.tensor_scalar(
                out=rhat, in0=dlt,
                scalar1=float(c1) * (2.0 ** 16), scalar2=float(r_lo),
                op0=OP.mult, op1=OP.add,
            )
            nc.vector.scalar_tensor_tensor(
                out=t1, in0=sq, scalar=-1.0, in1=rhat,
                op0=OP.mult, op1=OP.mult,
            )
            nc.vector.tensor_tensor(out=outv, in0=t1, in1=rq, op=OP.mult)
            nc.sync.dma_start(out=out_d[:, :], in_=outv)

    if not nc.is_finalized():
        nc.finalize()
    return nc


_NC = None
_NC_KEY = None


def _q_layout(x):
    # [C, T] fp32 -> [128, NT, KCH, 128] (partition, tile, chunk, token)
    return np.ascontiguousarray(
        x.reshape(KCH, 128, NT, 128).transpose(1, 2, 0, 3)
    ).astype(NP_IN)


def _i_layout(x):
    # [C, M] fp32 -> [128, 2, KCH, 2048] (partition, half, chunk, item)
    return np.ascontiguousarray(
        x.reshape(KCH, 128, 2, 2048).transpose(1, 2, 0, 3)
    ).astype(NP_IN)


def _run(queries, items, trace=False):
    global _NC, _NC_KEY
    queries = np.asarray(queries, dtype=np.float32)
    items = np.asarray(items, dtype=np.float32)

    i64 = items.astype(np.float64)
    n2 = np.einsum("mc,mc->m", i64, i64)
    r = 1.0 / np.sqrt(np.maximum(n2, 1e-24))
    r_lo, r_hi = float(r.min()), float(r.max())
    c1 = (r_hi - r_lo) / NCODE if r_hi > r_lo else 1.0
    code = np.clip(np.round((r - r_lo) / c1), 0, NCODE).astype(np.float32)
    nj = np.ascontiguousarray(
        np.broadcast_to((code * EPSC).astype(np.float16)[None, :], (128, M))
    )
    icat = _i_layout(np.ascontiguousarray(items.T).astype(np.float32))

    key = (r_lo, c1)
    if _NC is None or _NC_KEY != key:
        _NC = _build(r_lo, c1)
        _NC_KEY = key

    in_maps = []
    for b in range(NCORES):
        qcat = _q_layout(queries[b].T.astype(np.float32))
        q64 = queries[b].astype(np.float64)
        qn2 = np.einsum("tc,tc->t", q64, q64)
        rqn = (1.0 / np.sqrt(np.maximum(qn2, 1e-24))).astype(np.float32)
        in_maps.append({
            "q8": qcat,
            "i8": icat,
            "nj": nj,
            "rq": np.ascontiguousarray(rqn.reshape(NT, 128).T),
        })
    res = bass_utils.run_bass_kernel_spmd(
        _NC, in_maps, core_ids=list(range(NCORES)), trace=trace
    )
    out = np.stack([r["out"].T.reshape(T) for r in res.results]).astype(np.float32)
    return out, res.exec_time_ns


def kernel(queries, items):
    out, _ = _run(queries, items)
    return out


# revision 24
# speedup vs baseline: 1.0019x; 1.0019x over previous
import sys

sys.path.insert(0, "/opt/trn_rl_repo")

import numpy as np
import ml_dtypes
from concourse import bass, bacc, tile, bass_utils
from concourse.bass import mybir
from concourse.dve_spec import Spec, Src0, Src1, C0, lower as dve_lower, maxx
import concourse.dve_ops as dve_ops
from concourse.dve_uop import DveOpSpec

# Problem: queries (8, 2048, 512) f32, items (4096, 512) f32 -> (8, 2048) f32
#   score = q @ items.T ; j = argmax_m score[t, m] (softmax+top2 reduces to this)
#   out[t] = -score[t, j] / (||q_t|| * ||items_j||)
# Sharding: batch row b -> core b. Per core: T=2048 tokens, M=4096 items, C=512.
#
# Device pipeline per 128-token tile:
#   - matmuls accumulate scores into two 4-bank PSUM half-tiles
#     (fp8 e4m3 DoubleRow at 2 rows/cycle, or fp16 at 1 row/cycle).
#   - One custom DVE op per half reads PSUM directly:
#       out   = ((s + C0) - C0) + njit[m]      # quantize s to 2^-4 grid
#       accum = max(out)                        # packed running max
#     C0=3*2^18 pins the fp32 exponent so the +/- C0 round-trip snaps s onto a
#     uniform 2^-4 grid (Sterbenz-exact subtract); njit[m] = code[m]*2^-16
#     (code <= 2046 = quantized inverse item norm) rides in the freed low
#     mantissa bits; the packed sum is exactly representable in fp32.
#   - Tail decodes winner score and norm code from the single accumulated
#     value per token: out = -s_q * (r_lo + c1*code) / ||q||.
# No PSUM->SBUF copy pass exists; ScalarE is idle. Score-grid ties resolve
# deterministically to the largest norm code.

NCORES = 8
T = 2048
C = 512
M = 4096
NT = T // 128   # 16 token tiles
NB = M // 512   # 8 psum banks of 512 items
SPLIT = 1       # 0: fp16 matmul, 1: fp8 e4m3 DoubleRow matmul
KCH = 4         # contraction chunks of 128

F32 = mybir.dt.float32
F16 = mybir.dt.float16
F8 = mybir.dt.float8e4
OP = mybir.AluOpType
DR = mybir.MatmulPerfMode.DoubleRow

EPSC = 2.0 ** -16
NCODE = 2046
C0V = float(3 * 2 ** 18)

NP_F8 = ml_dtypes.float8_e4m3
IN_DT = F8 if SPLIT == 1 else F16
NP_IN = NP_F8 if SPLIT == 1 else np.float16


def _register_pack_max2():
    # accum_out = max over free dim of (grid-quantized in0 + in1).
    # Registered as a custom-DVE micro-op program carried in the NEFF DVE
    # table (the stock TENSOR_TENSOR_REDUCE opcode is fatal on this runtime).
    name = "PACK_MAX2_ANT"
    for existing in dve_ops.OPS:
        if existing.name == name:
            return existing

    def _ref(in0, in1, c0, c1, c2):
        q = (in0.astype(np.float32) + np.float32(c0)).astype(np.float32)
        q = (q - np.float32(c0)).astype(np.float32)
        o = q + in1.astype(np.float32)
        return o, np.maximum.reduce(o, axis=-1)

    spec = Spec(body=((Src0 + C0) - C0) + Src1, accum=maxx, reference=_ref)
    op = dve_ops.DveOp(name, spec, subdim=False, uops_sha={})
    dve_ops.OPS.append(op)
    dve_ops._SUB_OPCODE_FOR_NAME[name] = (
        dve_ops._CUSTOM_DVE_ROW_BASE + len(dve_ops.OPS) - 1
    )
    dve_ops.CUSTOM_DVE_SPECS[name] = spec
    for ver in ("v3", "v4"):
        uops = dve_lower(spec, ver=ver)
        op.uops_sha[ver] = DveOpSpec(
            name=name, opcode=dve_ops.get_dve_sub_opcode(name),
            uops=uops, rd1_en=True,
        ).sha(ver)
    return op


PACK_MAX2 = _register_pack_max2()


def _build(r_lo, c1):
    nc = bacc.Bacc()
    # tile-major q / half-major items: every DMA below moves contiguous
    # 512B-16KB runs per partition (few descriptors, low latency).
    q_d = nc.dram_tensor("q8", [128, NT, KCH, 128], IN_DT, kind="ExternalInput")
    i_d = nc.dram_tensor("i8", [128, 2, KCH, 2048], IN_DT, kind="ExternalInput")
    nj_d = nc.dram_tensor("nj", [128, M], F16, kind="ExternalInput")
    rq_d = nc.dram_tensor("rq", [128, NT], F32, kind="ExternalInput")
    out_d = nc.dram_tensor("out", [128, NT], F32, kind="ExternalOutput")

    with tile.TileContext(nc) as tc:
        with tc.tile_pool(name="big", bufs=1) as big, \
             tc.tile_pool(name="small", bufs=1) as small:

            it = big.tile([128, 2, KCH, 2048], IN_DT, name="it")
            qt = big.tile([128, NT, KCH, 128], IN_DT, name="qt")
            nj = big.tile([128, M], F16, name="nj")
            rq = small.tile([128, NT], F32, name="rq")
            dummy = small.tile([128, 1], F32, name="dummy")
            MA = small.tile([128, NT], F32, name="MA")
            MB = small.tile([128, NT], F32, name="MB")

            # DMA order: tile-0 q, items half 0 split by chunk pair
            # (kp0 matmuls start after the first 512KB), nj lo, items
            # half 1, nj hi, remaining q tiles
            nc.sync.dma_start(out=qt[:, 0], in_=q_d[:, 0])
            nc.sync.dma_start(out=it[:, 0, 0:2], in_=i_d[:, 0, 0:2])
            nc.sync.dma_start(out=it[:, 0, 2:4], in_=i_d[:, 0, 2:4])
            nc.sync.dma_start(out=nj[:, 0:2048], in_=nj_d[:, 0:2048])
            nc.sync.dma_start(out=it[:, 1, 0:2], in_=i_d[:, 1, 0:2])
            nc.sync.dma_start(out=it[:, 1, 2:4], in_=i_d[:, 1, 2:4])
            nc.sync.dma_start(out=nj[:, 2048:4096], in_=nj_d[:, 2048:4096])
            for i in range(1, NT):
                nc.sync.dma_start(out=qt[:, i], in_=q_d[:, i])
            nc.sync.dma_start(out=rq, in_=rq_d[:, :])

            with tc.tile_pool(name="bps", bufs=1, space="PSUM") as bps:
                # PE warmup: ~3.5us of throwaway matmuls issued while input
                # DMAs land, so the HAM clock-gate reaches 8/8 (2.4 GHz)
                # before the first real matmul. start=True on real tile-0
                # matmuls discards the garbage.
                wr = small.tile([128, 2, 512], IN_DT, name="wr")
                nc.gpsimd.memset(wr, 0)
                wl = wr[:, :, 0:128]
                warm = [
                    bps.tile([128, 2048], F32, tag=f"ps{h}", name="ps")
                    for h in range(2)
                ]
                for w in range(8):
                    nc.tensor.matmul(
                        warm[w // 4][:, bass.ts(w % 4, 512)],
                        wl if SPLIT == 1 else wl[:, 0, :],
                        wr[:, :, :] if SPLIT == 1 else wr[:, 0, :],
                        start=True, stop=True,
                        perf_mode=DR if SPLIT == 1 else None,
                    )
                for i in range(NT):
                    halves = [
                        bps.tile([128, 2048], F32, tag=f"ps{h}", name="ps")
                        for h in range(2)
                    ]
                    # half-A completes before half-B starts so pack-A can
                    # begin as early as possible
                    if SPLIT == 1:
                        for h in range(2):
                            for kp in range(KCH // 2):
                                for b in range(4):
                                    nc.tensor.matmul(
                                        halves[h][:, bass.ts(b, 512)],
                                        qt[:, i, 2 * kp:2 * kp + 2, :],
                                        it[:, h, 2 * kp:2 * kp + 2,
                                           bass.ts(b, 512)],
                                        start=(kp == 0),
                                        stop=(kp == KCH // 2 - 1),
                                        perf_mode=DR,
                                    )
                    else:
                        for h in range(2):
                            for k in range(KCH):
                                for b in range(4):
                                    nc.tensor.matmul(
                                        halves[h][:, bass.ts(b, 512)],
                                        qt[:, i, k, :],
                                        it[:, h, k, bass.ts(b, 512)],
                                        start=(k == 0), stop=(k == KCH - 1),
                                    )
                    nc.vector._custom_dve(
                        PACK_MAX2, out=dummy.broadcast_to([128, 2048]),
                        in0=halves[0], in1=nj[:, 0:2048], s0=C0V,
                        accum_out=MA[:, i:i + 1],
                    )
                    nc.vector._custom_dve(
                        PACK_MAX2, out=dummy.broadcast_to([128, 2048]),
                        in0=halves[1], in1=nj[:, 2048:4096], s0=C0V,
                        accum_out=MB[:, i:i + 1],
                    )

            # Decode: Mv = max(MA, MB); s_q = (Mv+C0)-C0; code = (Mv-s_q)*2^16
            # out = -s_q * (r_lo + c1*code) / ||q||
            Mv = small.tile([128, NT], F32, name="Mv")
            sq = small.tile([128, NT], F32, name="sq")
            dlt = small.tile([128, NT], F32, name="dlt")
            rhat = small.tile([128, NT], F32, name="rhat")
            t1 = small.tile([128, NT], F32, name="t1")
            outv = small.tile([128, NT], F32, name="outv")
            nc.vector.tensor_tensor(out=Mv, in0=MA, in1=MB, op=OP.max)
            nc.vector.tensor_scalar(
                out=sq, in0=Mv, scalar1=C0V, scalar2=C0V,
                op0=OP.add, op1=OP.subtract,
            )
            nc.vector.tensor_sub(dlt, Mv, sq)
            nc.vector.tensor_scalar(
                out=rhat, in0=dlt,
                scalar1=float(c1) * (2.0 ** 16), scalar2=float(r_lo),
                op0=OP.mult, op1=OP.add,
            )
            nc.vector.scalar_tensor_tensor(
                out=t1, in0=sq, scalar=-1.0, in1=rhat,
                op0=OP.mult, op1=OP.mult,
            )
            nc.vector.tensor_tensor(out=outv, in0=t1, in1=rq, op=OP.mult)
            nc.sync.dma_start(out=out_d[:, :], in_=outv)

    if not nc.is_finalized():
        nc.finalize()
    return nc


_NC = None
_NC_KEY = None


def _q_layout(x):
    # [C, T] fp32 -> [128, NT, KCH, 128] (partition, tile, chunk, token)
    return np.ascontiguousarray(
        x.reshape(KCH, 128, NT, 128).transpose(1, 2, 0, 3)
    ).astype(NP_IN)


def _i_layout(x):
    # [C, M] fp32 -> [128, 2, KCH, 2048] (partition, half, chunk, item)
    return np.ascontiguousarray(
        x.reshape(KCH, 128, 2, 2048).transpose(1, 2, 0, 3)
    ).astype(NP_IN)


def _run(queries, items, trace=False):
    global _NC, _NC_KEY
    queries = np.asarray(queries, dtype=np.float32)
    items = np.asarray(items, dtype=np.float32)

    i64 = items.astype(np.float64)
    n2 = np.einsum("mc,mc->m", i64, i64)
    r = 1.0 / np.sqrt(np.maximum(n2, 1e-24))
    r_lo, r_hi = float(r.min()), float(r.max())
    c1 = (r_hi - r_lo) / NCODE if r_hi > r_lo else 1.0
    code = np.clip(np.round((r - r_lo) / c1), 0, NCODE).astype(np.float32)
    nj = np.ascontiguousarray(
        np.broadcast_to((code * EPSC).astype(np.float16)[None, :], (128, M))
    )
    icat = _i_layout(np.ascontiguousarray(items.T).astype(np.float32))

    key = (r_lo, c1)
    if _NC is None or _NC_KEY != key:
        _NC = _build(r_lo, c1)
        _NC_KEY = key

    in_maps = []
    for b in range(NCORES):
        qcat = _q_layout(queries[b].T.astype(np.float32))
        q64 = queries[b].astype(np.float64)
        qn2 = np.einsum("tc,tc->t", q64, q64)
        rqn = (1.0 / np.sqrt(np.maximum(qn2, 1e-24))).astype(np.float32)
        in_maps.append({
            "q8": qcat,
            "i8": icat,
            "nj": nj,
            "rq": np.ascontiguousarray(rqn.reshape(NT, 128).T),
        })
    res = bass_utils.run_bass_kernel_spmd(
        _NC, in_maps, core_ids=list(range(NCORES)), trace=trace
    )
    out = np.stack([r["out"].T.reshape(T) for r in res.results]).astype(np.float32)
    return out, res.exec_time_ns


def kernel(queries, items):
    out, _ = _run(queries, items)
    return out
